# revision 19
# baseline (speedup 1.0000x reference)
"""Trainium2 Bass kernel for nn_DualLSTM: dual-LSTM scan + 2-layer FC head.

Strategy (8 NeuronCores, SPMD, no collectives):
  - Host: embedding gather, mask folding, gate-row permutation, fp16 casts.
  - Device A-stage: input-side gate terms A[t] = mask-sel(x_t) @ Wih_perm.T + b
    as batched matmuls -> DRAM (f32), consumed chunk-wise by the scan.
  - Device scan (replicated on all cores): 2048 sequential steps. Per step:
    128 stationary-weight matmuls (fp16, M=128 tiles of the 2x[2048,512]
    recurrent weights) -> gates in PSUM [i,f | g | o] so the elementwise
    chain starts before the burst ends; dual-branch c/h update with the
    mask select done algebraically via the identity
      d' = [cB2-cA1 | cA2-cB1] = [f_en | f_cn] * [cB1-c | cA1-c].
  - fc1 (replicated): hmidT = relu(fc1_W @ outs.T + b1) -> DRAM f16.
  - fc2 (V-sharded 1250 rows/core): outT_shard = fc2_Ws @ hmid + b2 -> f32.
  - Host: concat shards, transpose -> [2048, 10000] f32.

Gate-column layout (32 cols, each col = 128 gate rows):
  0-3 i_en | 4-7 i_cn | 8-11 f_en | 12-15 f_cn | 16-19 g_en | 20-23 g_cn
  | 24-27 o_en | 28-31 o_cn   (sigmoid on i,f,o; tanh on g)
"""

import os
import numpy as np
from contextlib import ExitStack

import concourse.bass as bass
import concourse.bacc as bacc
import concourse.mybir as mybir
import concourse.tile as tile
from concourse.ap import AP
from concourse.bass_utils import run_bass_kernel_spmd
from concourse.tile_rust import add_dep_helper

F16 = mybir.dt.float16
F32 = mybir.dt.float32
AF = mybir.ActivationFunctionType
OP = mybir.AluOpType
DS = bass.DynSlice

T, V, E, H = 2048, 10000, 256, 512
NCORES = 8
NCOLS = 32            # combined gate columns (2 cells x 16)
KC = H // 128         # 4 K-chunks for recurrent matvec
EK = 3                # K-chunks for A-stage (E=256 + bias col + pad -> 384)
VP = 10112            # V padded to 79*128 for fc1/hmid
MT1 = VP // 128       # 79 fc1 M-tiles
VSH = V // NCORES     # 1250 fc2 rows per core
VSP = 1280            # padded shard
MT2 = VSP // 128      # 10 fc2 M-tiles
CHUNK = 64            # scan steps per A-ring chunk
NCHUNK = T // CHUNK   # 32
UNROLL = 8            # steps per For_i body
NITER = CHUNK // UNROLL

# col blocks of 4: [i_en, i_cn, f_en, f_cn, g_en, g_cn, o_en, o_cn]
EN_COLS = frozenset(c for c in range(NCOLS) if (c // 4) % 2 == 0)


def _blocks2(t_ap: AP, b0: int, b1: int, bsize: int) -> AP:
    """[128, 2, bsize] AP over a [128, N] tile picking column blocks b0, b1
    (indices in units of bsize). b1 < b0 gives a negative stride."""
    pstep, pcount = t_ap.ap[0]
    return AP(t_ap.tensor, t_ap.offset + b0 * bsize,
              [[pstep, pcount], [(b1 - b0) * bsize, 2], [1, bsize]])


def _v24(t_ap: AP) -> AP:
    """View a [128, 8] slice as [128, 2, 4]."""
    return t_ap.rearrange("p (b c) -> p b c", c=4)


def build_program():
    nc = bacc.Bacc("TRN2", target_bir_lowering=False, debug=False,
                   num_devices=NCORES)

    # ---- DRAM I/O ----
    wsb_d = nc.dram_tensor("wsb", [128, NCOLS * KC * 128], F16, kind="ExternalInput")
    wih_d = nc.dram_tensor("wih", [128, NCOLS * EK * 128], F16, kind="ExternalInput")
    xte_d = nc.dram_tensor("xte", [128, EK * T], F16, kind="ExternalInput")
    xtc_d = nc.dram_tensor("xtc", [128, EK * T], F16, kind="ExternalInput")
    mcol_d = nc.dram_tensor("mcol", [128, T + 2 * CHUNK], F32, kind="ExternalInput")
    mncol_d = nc.dram_tensor("mncol", [128, T + 2 * CHUNK], F32, kind="ExternalInput")
    w1t_d = nc.dram_tensor("w1t", [H, VP], F16, kind="ExternalInput")
    b1c_d = nc.dram_tensor("b1c", [128, MT1], F32, kind="ExternalInput")
    w2t_d = nc.dram_tensor("w2t", [VP, VSP], F16, kind="ExternalInput")
    b2c_d = nc.dram_tensor("b2c", [128, MT2], F32, kind="ExternalInput")
    outT_d = nc.dram_tensor("outT", [VSP, T], F32, kind="ExternalOutput")

    # internal DRAM scratch
    # a_scratch element (ch, p, c*CHUNK + tl) = A[gate row c*128+p, t=ch*64+tl]
    a_dram = nc.dram_tensor("a_scratch", [NCHUNK + 2, 128, CHUNK * NCOLS], F32)
    hmid_d = nc.dram_tensor("hmid_scratch", [VP, T], F16)

    with tile.TileContext(nc) as tc, ExitStack() as stk:
        # ---------- persistent SBUF (whole program) ----------
        pers = stk.enter_context(tc.tile_pool(name="pers", bufs=1))
        outsT = pers.tile([128, 4 * T], F16, tag="outsT")   # col = k*T + t
        b1c = pers.tile([128, MT1], F32, tag="b1c")
        b2c = pers.tile([128, MT2], F32, tag="b2c")

        def ptile(nm, shape, dt):
            return [pers.tile(shape, dt, tag=f"{nm}{i}", name=f"{nm}{i}")
                    for i in range(2)]

        # scan state / temps (parity pairs)
        hbf = ptile("hbf", [128, 8], F16)
        # CU = [c | c | cA1 | cB1]
        CU = ptile("CU", [128, 16], F32)
        S1 = ptile("S1", [128, 24], F32)    # sig(i), sig(f), tanh(g)
        S2 = ptile("S2", [128, 8], F32)     # sig(o)
        G1I = ptile("G1I", [128, 16], F32)
        G1G = ptile("G1G", [128, 8], F32)
        G1O = ptile("G1O", [128, 8], F32)
        PQ = ptile("PQ", [128, 8], F32)
        T5 = ptile("T5", [128, 8], F32)
        DZ = ptile("DZ", [128, 8], F32)
        DP = ptile("DP", [128, 8], F32)
        SEL = ptile("SEL", [128, 8], F32)
        TH = ptile("TH", [128, 8], F32)
        D2 = ptile("D2", [128, 4], F32)

        nc.sync.dma_start(b1c[:], b1c_d[:])
        nc.sync.dma_start(b2c[:], b2c_d[:])
        nc.gpsimd.memset(hbf[0][:], 0.0)
        nc.gpsimd.memset(CU[0][:], 0.0)

        # ---------- scan-phase SBUF (freed before fc2) ----------
        with tc.tile_pool(name="scanp", bufs=1) as scanp:
            wsb = scanp.tile([128, NCOLS * KC * 128], F16, tag="wsb")
            aring = [scanp.tile([128, CHUNK * NCOLS], F32, tag=f"aring{i}",
                                name=f"aring{i}") for i in range(2)]
            nc.sync.dma_start(wsb[:], wsb_d[:])

            # ---------- A-stage (tb outer so early chunks finish first) ----
            with tc.tile_pool(name="astg", bufs=1) as ast_pool, \
                 tc.tile_pool(name="axts", bufs=1) as xts_pool, \
                 tc.tile_pool(name="apsum", bufs=4, space="PSUM") as aps_pool, \
                 tc.tile_pool(name="acopy", bufs=4) as acp_pool:
                wih = ast_pool.tile([128, NCOLS * EK * 128], F16, tag="wih")
                nc.sync.dma_start(wih[:], wih_d[:])
                xte = xts_pool.tile([128, EK * T], F16, tag="xte")
                xtc = xts_pool.tile([128, EK * T], F16, tag="xtc")
                nc.sync.dma_start(xte[:], xte_d[:])
                nc.sync.dma_start(xtc[:], xtc_d[:])
                for tb in range(4):  # t-blocks of 512
                    for c in range(NCOLS):
                        xts = xte if c in EN_COLS else xtc
                        ps = aps_pool.tile([128, 512], F32, tag="aps")
                        for kx in range(EK):
                            nc.tensor.matmul(
                                ps[:],
                                wih[:, (c * EK + kx) * 128:
                                    (c * EK + kx + 1) * 128],
                                xts[:, kx * T + tb * 512:
                                    kx * T + tb * 512 + 512],
                                start=(kx == 0), stop=(kx == EK - 1))
                        stg = acp_pool.tile([128, 512], F32, tag="astg")
                        nc.vector.tensor_copy(stg[:], ps[:])
                        # dst: (ch=tb*8+nhi, p, c*CHUNK + nlo)
                        dst = AP(a_dram,
                                 tb * 8 * (128 * CHUNK * NCOLS) + c * CHUNK,
                                 [[CHUNK * NCOLS, 128],
                                  [128 * CHUNK * NCOLS, 8],
                                  [1, CHUNK]])
                        src = stg[:].rearrange("p (a b) -> p a b", a=8)
                        nc.sync.dma_start(dst, src)

            # ---------- scan ----------
            mcB = scanp.tile([128, 2 * CHUNK], F32, tag="mcB")
            mnB = scanp.tile([128, 2 * CHUNK], F32, tag="mnB")
            outsS = scanp.tile([128, 4 * 2 * CHUNK], F16, tag="outsS")
            NBODY = NCHUNK // 2
            adv = a_dram[:]
            # preload body 0 inputs
            nc.sync.dma_start(aring[0][:], a_dram[0])
            nc.sync.dma_start(aring[1][:], a_dram[1])
            nc.sync.dma_start(mcB[:], mcol_d[:, 0:2 * CHUNK])
            nc.sync.dma_start(mnB[:], mncol_d[:, 0:2 * CHUNK])
            with tc.tile_pool(name="psi", bufs=2, space="PSUM") as psi_pool, \
                 tc.tile_pool(name="psf", bufs=2, space="PSUM") as psf_pool, \
                 tc.tile_pool(name="psG", bufs=2, space="PSUM") as psG_pool, \
                 tc.tile_pool(name="psO", bufs=2, space="PSUM") as psO_pool:
                with tc.For_i(0, NBODY, 1,
                              hint_engines=(mybir.EngineType.PE,)) as ib:
                    for sl2 in range(2 * CHUNK):
                        ci_loc, sl = sl2 // CHUNK, sl2 % CHUNK
                        if True:
                            slot = ci_loc
                            tstep = sl2          # parity/static col index
                            par, nxt = tstep % 2, 1 - (tstep % 2)
                            psi = psi_pool.tile([128, 8], F32, tag="psi")
                            psf = psf_pool.tile([128, 8], F32, tag="psf")
                            psG = psG_pool.tile([128, 8], F32, tag="psG")
                            psO = psO_pool.tile([128, 8], F32, tag="psO")
                            for c in (list(range(16, 24))
                                      + list(range(8, 16))
                                      + list(range(0, 8))
                                      + list(range(24, 32))):
                                hoff = 0 if c in EN_COLS else 4
                                if c < 8:
                                    pst = psi[:, c:c + 1]
                                elif c < 16:
                                    pst = psf[:, c - 8:c - 7]
                                elif c < 24:
                                    pst = psG[:, c - 16:c - 15]
                                else:
                                    pst = psO[:, c - 24:c - 23]
                                for k in range(KC):
                                    nc.tensor.matmul(
                                        pst,
                                        wsb[:, (c * KC + k) * 128:
                                            (c * KC + k + 1) * 128],
                                        hbf[par][:, hoff + k:hoff + k + 1],
                                        start=(k == 0), stop=(k == KC - 1))
                            av = aring[slot][:].rearrange(
                                "p (c t) -> p c t", c=NCOLS)
                            mm = mcB[:, sl2:sl2 + 1]
                            mn = mnB[:, sl2:sl2 + 1]
                            cu = CU[par]
                            # ---- DVE/ACT chain (emission = FIFO order) ----
                            nc.vector.tensor_tensor(
                                G1G[par][:], psG[:],
                                av[:, 16:24, sl:sl + 1].squeeze(2), OP.add)
                            nc.scalar.activation(S1[par][:, 16:24], G1G[par][:],
                                                 AF.Tanh)
                            nc.vector.tensor_tensor(
                                G1I[par][:, 8:16], psf[:],
                                av[:, 8:16, sl:sl + 1].squeeze(2), OP.add)
                            nc.scalar.activation(S1[par][:, 8:16],
                                                 G1I[par][:, 8:16], AF.Sigmoid)
                            nc.vector.tensor_tensor(
                                T5[par][:], _v24(S1[par][:, 8:16]),
                                _v24(cu[:, 0:8]), OP.mult)
                            nc.vector.tensor_tensor(
                                G1I[par][:, 0:8], psi[:],
                                av[:, 0:8, sl:sl + 1].squeeze(2), OP.add)
                            nc.scalar.activation(S1[par][:, 0:8],
                                                 G1I[par][:, 0:8], AF.Sigmoid)
                            nc.vector.tensor_tensor(
                                PQ[par][:], _v24(S1[par][:, 0:8]),
                                _v24(S1[par][:, 16:24]), OP.mult)
                            # u2a = [cA1|cB1] -> CU[8:16]
                            nc.vector.tensor_tensor(
                                _v24(cu[:, 8:16]), _v24(T5[par][:]),
                                _v24(PQ[par][:]), OP.add)
                            # o-gate path: keep out of the critical chain
                            g1o_inst = nc.vector.tensor_tensor(
                                G1O[par][:], psO[:],
                                av[:, 24:32, sl:sl + 1].squeeze(2), OP.add)
                            nc.scalar.activation(S2[par][:], G1O[par][:],
                                                 AF.Sigmoid)
                            # dz = [cB1-c | cA1-c]
                            nc.vector.tensor_tensor(
                                _v24(DZ[par][:]), _blocks2(cu[:], 3, 2, 4),
                                _v24(cu[:, 0:8]), OP.subtract)
                            # dp = [f_en|f_cn]*dz = [cB2-cA1 | cA2-cB1]
                            dp_inst = nc.vector.tensor_tensor(
                                _v24(DP[par][:]), _v24(S1[par][:, 8:16]),
                                _v24(DZ[par][:]), OP.mult)
                            # ce = cA1 + (1-m)*(cB2-cA1); cc = cB1 + m*(cA2-cB1)
                            add_dep_helper(g1o_inst.ins, dp_inst.ins,
                                           sync=False,
                                           reason="o-gate add out of chain")
                            nc.vector.scalar_tensor_tensor(
                                SEL[par][:, 0:4], DP[par][:, 0:4], mn,
                                cu[:, 8:12], OP.mult, OP.add)
                            nc.vector.scalar_tensor_tensor(
                                SEL[par][:, 4:8], DP[par][:, 4:8], mm,
                                cu[:, 12:16], OP.mult, OP.add)
                            nc.scalar.activation(TH[par][:], SEL[par][:],
                                                 AF.Tanh)
                            # h = o * tanh(c*)  (f16 out, feeds next matmuls)
                            nc.vector.tensor_tensor(
                                hbf[nxt][:], S2[par][:], TH[par][:], OP.mult)
                            # c' = ce + m*(cc-ce) -> CU[nxt][0:8] (duplicated)
                            nc.vector.tensor_tensor(
                                D2[par][:], SEL[par][:, 4:8], SEL[par][:, 0:4],
                                OP.subtract)
                            nc.vector.scalar_tensor_tensor(
                                CU[nxt][:, 0:4], D2[par][:], mm,
                                SEL[par][:, 0:4], OP.mult, OP.add)
                            nc.vector.tensor_copy(CU[nxt][:, 4:8],
                                                  CU[nxt][:, 0:4])
                            # outs[t] = h_en + h_cn -> outsT cols k*T + t
                            ovS = outsS[:].rearrange(
                                "p (k t) -> p k t", k=4)
                            nc.vector.tensor_tensor(
                                ovS[:, :, sl2:sl2 + 1].squeeze(2),
                                hbf[nxt][:, 0:4], hbf[nxt][:, 4:8], OP.add)
                    # ---- body tail: flush outs, prefetch next body ----
                    ovT = outsT[:].rearrange("p (k t) -> p k t", k=4)
                    nc.sync.dma_start(
                        ovT[:, :, DS(ib * (2 * CHUNK), 2 * CHUNK)],
                        outsS[:].rearrange("p (k t) -> p k t", k=4))
                    nc.sync.dma_start(
                        aring[0][:], adv[DS(2 * ib + 2, 1)].squeeze(0))
                    nc.sync.dma_start(
                        aring[1][:], adv[DS(2 * ib + 3, 1)].squeeze(0))
                    nc.sync.dma_start(
                        mcB[:], mcol_d[:, DS((ib + 1) * (2 * CHUNK),
                                             2 * CHUNK)])
                    nc.sync.dma_start(
                        mnB[:], mncol_d[:, DS((ib + 1) * (2 * CHUNK),
                                              2 * CHUNK)])

        # ---------- fc1 ----------
        with tc.tile_pool(name="f1w", bufs=1) as f1w, \
             tc.tile_pool(name="f1ps", bufs=4, space="PSUM") as f1ps, \
             tc.tile_pool(name="f1st", bufs=4) as f1st:
            w1sb = f1w.tile([128, 4 * VP], F16, tag="w1sb")
            for k in range(4):
                nc.sync.dma_start(w1sb[:, k * VP:(k + 1) * VP],
                                  w1t_d[k * 128:(k + 1) * 128, :])
            for nb in range(4):
                for m in range(MT1):
                    ps = f1ps.tile([128, 512], F32, tag="f1p")
                    for k in range(4):
                        nc.tensor.matmul(
                            ps[:],
                            w1sb[:, k * VP + m * 128: k * VP + m * 128 + 128],
                            outsT[:, k * T + nb * 512: k * T + nb * 512 + 512],
                            start=(k == 0), stop=(k == 3))
                    hst = f1st.tile([128, 512], F16, tag="f1h")
                    nc.scalar.activation(hst[:], ps[:], AF.Relu,
                                         bias=b1c[:, m:m + 1])
                    nc.sync.dma_start(
                        hmid_d[m * 128:(m + 1) * 128, nb * 512:(nb + 1) * 512],
                        hst[:])

        # ---------- fc2 ----------
        MGROUPS = [(0, 4), (4, 4), (8, 2)]
        with tc.tile_pool(name="hblk", bufs=MT1) as hbp, \
             tc.tile_pool(name="w2p", bufs=6) as w2p, \
             tc.tile_pool(name="f2ps", bufs=5, space="PSUM") as f2ps, \
             tc.tile_pool(name="f2st", bufs=4) as f2st:
            for nb in range(4):
                hts = []
                for k2 in range(MT1):
                    ht = hbp.tile([128, 512], F16, tag="hblk")
                    nc.sync.dma_start(
                        ht[:], hmid_d[k2 * 128:(k2 + 1) * 128,
                                      nb * 512:(nb + 1) * 512])
                    hts.append(ht)
                for (m0, mw) in MGROUPS:
                    pss = [f2ps.tile([128, 512], F32, tag="f2p", name="f2p")
                           for _ in range(mw)]
                    for k2 in range(MT1):
                        w2c = w2p.tile([128, 512], F16, tag="w2c")
                        nc.sync.dma_start(
                            w2c[:, 0:mw * 128],
                            w2t_d[k2 * 128:(k2 + 1) * 128,
                                  m0 * 128: m0 * 128 + mw * 128])
                        for mi in range(mw):
                            nc.tensor.matmul(
                                pss[mi][:], w2c[:, mi * 128:(mi + 1) * 128],
                                hts[k2][:],
                                start=(k2 == 0), stop=(k2 == MT1 - 1))
                    for mi in range(mw):
                        m = m0 + mi
                        ost = f2st.tile([128, 512], F32, tag="f2o")
                        nc.scalar.activation(ost[:], pss[mi][:], AF.Identity,
                                             bias=b2c[:, m:m + 1])
                        nc.sync.dma_start(
                            outT_d[m * 128:(m + 1) * 128,
                                   nb * 512:(nb + 1) * 512], ost[:])

    nc.compile()
    return nc


# ---------------- host side ----------------

_NC_CACHE = {}


def _get_program():
    if "nc" not in _NC_CACHE:
        _NC_CACHE["nc"] = build_program()
    return _NC_CACHE["nc"]


def _build_big(Wen, Wcn):
    """Stack two cells' torch-gate-order rows [i,f,g,o] into combined
    [i_en, i_cn, f_en, f_cn, g_en, g_cn, o_en, o_cn] order."""
    blocks = []
    for gi in range(4):
        blocks.append(Wen[gi * H:(gi + 1) * H])
        blocks.append(Wcn[gi * H:(gi + 1) * H])
    return np.concatenate(blocks, axis=0)


def _pack_lhsT(bigw, nk):
    """[4096, nk*128] -> [128, 32*nk*128] with tile (c,k) at col
    (c*nk+k)*128 + m, element [p] = bigw[c*128+m, k*128+p]."""
    arr = bigw.reshape(NCOLS, 128, nk, 128)           # [c, m, k, p]
    return np.ascontiguousarray(arr.transpose(3, 0, 2, 1)
                                ).reshape(128, NCOLS * nk * 128)


def host_prep(inputs):
    tok = np.asarray(inputs["token_ids"]).astype(np.int64)
    msk = np.asarray(inputs["mask"]).astype(np.float32)
    emb = np.asarray(inputs["emb"], dtype=np.float32)
    f32 = lambda n: np.asarray(inputs[n], dtype=np.float32)
    Wih_en, Whh_en = f32("Wih_en"), f32("Whh_en")
    bih_en, bhh_en = f32("bih_en"), f32("bhh_en")
    Wih_cn, Whh_cn = f32("Wih_cn"), f32("Whh_cn")
    bih_cn, bhh_cn = f32("bih_cn"), f32("bhh_cn")
    fc1_W, fc1_b = f32("fc1_W"), f32("fc1_b")
    fc2_W, fc2_b = f32("fc2_W"), f32("fc2_b")

    # --- scan recurrent weights ---
    bigwhh = _build_big(Whh_en, Whh_cn)               # [4096, 512]
    wsb = _pack_lhsT(bigwhh, KC).astype(np.float16)

    # --- A-stage weights: [Wih | b | 0] augmented to K=384 ---
    def aug(Wih, b):
        return np.concatenate(
            [Wih, b[:, None],
             np.zeros((4 * H, EK * 128 - E - 1), np.float32)], axis=1)
    ae = aug(Wih_en, bih_en + bhh_en)                 # [2048, 384]
    ac = aug(Wih_cn, bih_cn + bhh_cn)
    bigwih = _build_big(ae, ac)                       # [4096, 384]
    wih = _pack_lhsT(bigwih, EK).astype(np.float16)

    # --- X augmented, mask-folded, transposed ---
    X = emb[tok]                                      # [T, E]
    ones = np.ones((T, 1), np.float32)
    zpad = np.zeros((T, EK * 128 - E - 1), np.float32)
    xa_en = np.concatenate([X * msk[:, None], ones, zpad], axis=1)
    xa_cn = np.concatenate([X * (1.0 - msk)[:, None], ones, zpad], axis=1)
    # xts[p, kx*T + t] = xa[t, kx*128+p]
    xte = np.ascontiguousarray(
        xa_en.reshape(T, EK, 128).transpose(2, 1, 0)).reshape(128, EK * T)
    xtc = np.ascontiguousarray(
        xa_cn.reshape(T, EK, 128).transpose(2, 1, 0)).reshape(128, EK * T)

    mpad = np.concatenate([msk, np.zeros(2 * CHUNK, np.float32)])
    mcol = np.ascontiguousarray(
        np.broadcast_to(mpad[None, :], (128, T + 2 * CHUNK)))
    mncol = np.ascontiguousarray(
        np.broadcast_to((1.0 - mpad)[None, :], (128, T + 2 * CHUNK)))

    # --- fc1 ---
    w1p = np.zeros((VP, H), np.float32)
    w1p[:V] = fc1_W
    w1t = np.ascontiguousarray(w1p.T).astype(np.float16)   # [512, VP]
    b1p = np.zeros((VP,), np.float32)
    b1p[:V] = fc1_b
    b1c = np.ascontiguousarray(b1p.reshape(MT1, 128).T)    # [128, MT1]

    # --- fc2 shards ---
    shard_w, shard_b = [], []
    for s in range(NCORES):
        w2p_ = np.zeros((VSP, VP), np.float32)
        w2p_[:VSH, :V] = fc2_W[s * VSH:(s + 1) * VSH]
        shard_w.append(np.ascontiguousarray(w2p_.T).astype(np.float16))
        b2p = np.zeros((VSP,), np.float32)
        b2p[:VSH] = fc2_b[s * VSH:(s + 1) * VSH]
        shard_b.append(np.ascontiguousarray(b2p.reshape(MT2, 128).T))

    common = dict(wsb=wsb, wih=wih, xte=xte.astype(np.float16),
                  xtc=xtc.astype(np.float16), mcol=mcol, mncol=mncol,
                  w1t=w1t, b1c=b1c)
    in_maps = []
    for s in range(NCORES):
        m = dict(common)
        m["w2t"] = shard_w[s]
        m["b2c"] = shard_b[s]
        in_maps.append(m)
    return in_maps


LAST_RESULT = None


def _install_ntff_shim():
    """The agent image lacks antenv.axon_hooks; register the ctypes NTFF
    profiling hook manually so trace=True works."""
    import sys, types
    if "antenv.axon_hooks" in sys.modules:
        return
    import antenv
    mod = types.ModuleType("antenv.axon_hooks")
    _h = [None]
    mod.set_axon_ntff_profile_hook = lambda h: _h.__setitem__(0, h)
    mod.get_axon_ntff_profile_hook = lambda: _h[0]
    sys.modules["antenv.axon_hooks"] = mod
    antenv.axon_hooks = mod
    from trn_agent_boot.trn_boot import _ntff_profile_via_ctypes
    mod.set_axon_ntff_profile_hook(
        _ntff_profile_via_ctypes("/opt/axon/libaxon_pjrt.so"))


def kernel(**inputs):
    global LAST_RESULT
    trace = bool(os.environ.get("DUALLSTM_TRACE"))
    if trace:
        _install_ntff_shim()
    nc = _get_program()
    in_maps = host_prep(inputs)
    res = run_bass_kernel_spmd(nc, in_maps, core_ids=list(range(NCORES)),
                               trace=trace)
    LAST_RESULT = res
    out = np.empty((T, V), np.float32)
    for s in range(NCORES):
        out[:, s * VSH:(s + 1) * VSH] = res.results[s]["outT"][:VSH].T
    return out
